# revision 1
# baseline (speedup 1.0000x reference)
"""Trainium2 Bass kernel: 2-layer LSTM (SEQ=1024, B=64, IN=H=512).

Sharding: data-parallel over batch across 8 NeuronCores (8 rows each);
weights replicated. Per core the kernel runs a chunked pipeline:

  - x-part gate pre-activations Z0x = W0x @ x^T + b0 are computed in bulk
    per chunk of T=16 timesteps (moving operand = x^T chunk, full PE width).
  - the sequential recurrence computes z^T = W_h @ h^T per step with the
    weight tiles stationary and h^T (128x8) moving, so the gate
    pre-activations land transposed ([gate, batch]) in PSUM, which keeps
    all elementwise work on 128 partitions.
  - layer-1's x-part (W1x @ h0) is bulk-computed per chunk as soon as
    layer 0 finishes the chunk; layer 1 runs one chunk behind layer 0 so
    its matmuls fill the PE while layer 0's elementwise chain runs.
  - h1 chunks are transposed back via the PE and DMA'd out contiguously.

All matmul operands are bf16 (fp32 PSUM accumulation); the cell state and
all elementwise math stay fp32.
"""

import sys

if "/opt/trn_rl_repo" not in sys.path:
    sys.path.insert(0, "/opt/trn_rl_repo")

import numpy as np
import ml_dtypes

import concourse.bass as bass
import concourse.bacc as bacc
import concourse.mybir as mybir
from concourse.tile import TileContext
from concourse.masks import make_identity
from concourse.bass import ts
from concourse.bass_utils import run_bass_kernel_spmd

SEQ, B, IN, H = 1024, 64, 512, 512
NCORES = 8
BL = B // NCORES          # batch rows per core
T = 16                    # timesteps per chunk
G = 4 * H                 # stacked gate dim (i,f,o,g)
KC = IN // 128            # contraction chunks of 128
MC = G // 128             # gate-dim chunks of 128
TB = T * BL               # columns per chunk (t-major, batch minor)
F32 = mybir.dt.float32
BF16 = mybir.dt.bfloat16
BFNP = ml_dtypes.bfloat16
AF = mybir.ActivationFunctionType


def build_program(seq=SEQ):
    nch = seq // T
    nc = bacc.Bacc("TRN2", target_bir_lowering=False, debug=False,
                   num_devices=NCORES)

    xtb_d = nc.dram_tensor("xtb", [(nch + 1) * 128, KC, TB], BF16,
                           kind="ExternalInput")
    w_d = {
        name: nc.dram_tensor(name, [128, KC, G], BF16, kind="ExternalInput")
        for name in ("w0x", "w0h", "w1x", "w1h")
    }
    b0_d = nc.dram_tensor("b0r", [1, G], BF16, kind="ExternalInput")
    b1_d = nc.dram_tensor("b1r", [1, G], BF16, kind="ExternalInput")
    out_d = nc.dram_tensor("out", [seq * BL, H], F32, kind="ExternalOutput")
    fin_d = nc.dram_tensor("fin", [4 * 128, KC, BL], F32, kind="ExternalOutput")

    with TileContext(nc) as tc:
        with (
            tc.tile_pool(name="pers", bufs=1) as pers,
            tc.tile_pool(name="xtp", bufs=2) as xtp,
            tc.tile_pool(name="ewp", bufs=2) as ewp,
            tc.tile_pool(name="zstp", bufs=2, space="PSUM") as zstp,
            tc.tile_pool(name="blkp", bufs=1, space="PSUM") as blkp,
            tc.tile_pool(name="tpp", bufs=1, space="PSUM") as tpp,
            tc.tile_pool(name="outbp", bufs=2) as outbp,
        ):
            # --- persistent SBUF state ---
            w_s = {}
            for name in ("w0x", "w0h", "w1x", "w1h"):
                w = pers.tile([128, KC, G], BF16, name=name + "_s")
                nc.sync.dma_start(out=w, in_=w_d[name].ap())
                w_s[name] = w
            b0r = pers.tile([1, G], BF16)
            nc.sync.dma_start(out=b0r, in_=b0_d.ap())
            b1r = pers.tile([1, G], BF16)
            nc.sync.dma_start(out=b1r, in_=b1_d.ap())
            ones = pers.tile([1, TB], BF16)
            nc.gpsimd.memset(ones, 1.0)
            ident = pers.tile([128, 128], F32)
            make_identity(nc, ident)

            z0x = pers.tile([128, MC, TB], F32)   # layer0 x-part + bias, chunk
            z1x = pers.tile([128, MC, TB], F32)   # layer1 x-part + bias, chunk
            # rolling h^T chunks (bf16, slot 0 = carry from previous chunk)
            h0r = pers.tile([128, KC, (T + 1) * BL], BF16)
            nc.gpsimd.memset(h0r, 0.0)
            h1r = pers.tile([128, KC, (T + 1) * BL], BF16)
            nc.gpsimd.memset(h1r, 0.0)
            h1c = pers.tile([128, KC, TB], F32)   # layer1 h chunk (fp32, for out)
            h0f = pers.tile([128, KC, BL], F32)   # layer0 h state (fp32)
            nc.gpsimd.memset(h0f, 0.0)
            c0 = pers.tile([128, KC, BL], F32)
            nc.gpsimd.memset(c0, 0.0)
            c1 = pers.tile([128, KC, BL], F32)
            nc.gpsimd.memset(c1, 0.0)

            def load_xt(ci):
                xt = xtp.tile([128, KC, TB], BF16, tag="xt", name="xt")
                nc.sync.dma_start(out=xt, in_=xtb_d.ap()[ts(ci, 128), :, :])
                return xt

            def bulk(zx, w, bias_r, mov):
                # zx[:, m, n] = sum_k w[k, m] * mov[k, n] + bias[m]
                for half in range(2):
                    bp = blkp.tile([128, 8, TB], F32, tag="bp", name="bp")
                    for m8 in range(8):
                        m = half * 8 + m8
                        for k in range(KC):
                            nc.tensor.matmul(
                                bp[:, m8, :],
                                lhsT=w[:, k, m * 128:(m + 1) * 128],
                                rhs=mov[:, k, :],
                                start=(k == 0), stop=False)
                        nc.tensor.matmul(
                            bp[:, m8, :],
                            lhsT=bias_r[0:1, m * 128:(m + 1) * 128],
                            rhs=ones[0:1, :],
                            start=False, stop=True)
                    nc.vector.tensor_copy(zx[:, half * 8:(half + 1) * 8, :], bp)

            def cell(layer, t):
                wh = w_s["w0h"] if layer == 0 else w_s["w1h"]
                hr = h0r if layer == 0 else h1r
                zx = z0x if layer == 0 else z1x
                cst = c0 if layer == 0 else c1
                zp = zstp.tile([128, MC, BL], F32, tag=f"zp{layer}", name="zp")
                for m in range(MC):
                    for k in range(KC):
                        nc.tensor.matmul(
                            zp[:, m, :],
                            lhsT=wh[:, k, m * 128:(m + 1) * 128],
                            rhs=hr[:, k, t * BL:(t + 1) * BL],
                            start=(k == 0), stop=(k == KC - 1))
                zf = ewp.tile([128, MC, BL], F32, tag=f"zf{layer}", name="zf")
                nc.vector.tensor_add(zf, zp, zx[:, :, t * BL:(t + 1) * BL])
                sg = ewp.tile([128, 12, BL], F32, tag=f"sg{layer}", name="sg")
                nc.scalar.activation(sg, zf[:, 0:12, :], AF.Sigmoid)
                tg = ewp.tile([128, KC, BL], F32, tag=f"tg{layer}", name="tg")
                nc.scalar.activation(tg, zf[:, 12:16, :], AF.Tanh)
                ig = ewp.tile([128, KC, BL], F32, tag=f"ig{layer}", name="ig")
                nc.vector.tensor_mul(ig, sg[:, 0:4, :], tg)
                fc = ewp.tile([128, KC, BL], F32, tag=f"fc{layer}", name="fc")
                nc.vector.tensor_mul(fc, sg[:, 4:8, :], cst)
                nc.vector.tensor_add(cst, ig, fc)
                th = ewp.tile([128, KC, BL], F32, tag=f"th{layer}", name="th")
                nc.scalar.activation(th, cst, AF.Tanh)
                if layer == 0:
                    nc.vector.tensor_mul(h0f, sg[:, 8:12, :], th)
                    nc.vector.tensor_copy(hr[:, :, (t + 1) * BL:(t + 2) * BL], h0f)
                else:
                    hslice = h1c[:, :, t * BL:(t + 1) * BL]
                    nc.vector.tensor_mul(hslice, sg[:, 8:12, :], th)
                    nc.vector.tensor_copy(hr[:, :, (t + 1) * BL:(t + 2) * BL],
                                          hslice)

            def roll(hr):
                nc.vector.tensor_copy(hr[:, :, 0:BL],
                                      hr[:, :, T * BL:(T + 1) * BL])

            def flush_out(ci):
                tp = tpp.tile([128, H], F32, tag="tp", name="tp")
                for k in range(KC):
                    nc.tensor.transpose(tp[:, k * 128:(k + 1) * 128],
                                        h1c[:, k, :], ident)
                ob = outbp.tile([128, H], F32, tag="ob", name="ob")
                nc.vector.tensor_copy(ob, tp)
                nc.sync.dma_start(out=out_d.ap()[ts(ci, 128), :], in_=ob)

            # --- prologue: chunk 0 of layer 0 ---
            xt = load_xt(0)
            bulk(z0x, w_s["w0x"], b0r, xt)
            for t in range(T):
                cell(0, t)
            bulk(z1x, w_s["w1x"], b1r, h0r[:, :, BL:])
            roll(h0r)
            xt = load_xt(1)
            bulk(z0x, w_s["w0x"], b0r, xt)

            # --- main loop: layer0 chunk c interleaved with layer1 chunk c-1 ---
            with tc.For_i(1, nch) as c:
                xt = load_xt(c + 1)
                for t in range(T):
                    cell(0, t)
                    cell(1, t)
                flush_out(c - 1)
                bulk(z1x, w_s["w1x"], b1r, h0r[:, :, BL:])
                roll(h0r)
                roll(h1r)
                bulk(z0x, w_s["w0x"], b0r, xt)

            # --- epilogue: layer 1, last chunk ---
            for t in range(T):
                cell(1, t)
            flush_out(nch - 1)
            nc.sync.dma_start(out=fin_d.ap()[0:128, :, :], in_=h0f)
            nc.sync.dma_start(out=fin_d.ap()[128:256, :, :],
                              in_=h1c[:, :, (T - 1) * BL:T * BL])
            nc.sync.dma_start(out=fin_d.ap()[256:384, :, :], in_=c0)
            nc.sync.dma_start(out=fin_d.ap()[384:512, :, :], in_=c1)

    nc.compile()
    return nc


def _wprep(wpart):
    # [G, 512] -> [p, kc, m] with w[p, kc, m] = wpart[m, kc*128+p]
    g = wpart.shape[0]
    return np.ascontiguousarray(
        wpart.T.reshape(KC, 128, g).transpose(1, 0, 2)).astype(BFNP)


def prep_inputs(x, W0, b0, W1, b1, seq=SEQ):
    nch = seq // T
    x = np.asarray(x, np.float32)
    W0 = np.asarray(W0, np.float32)
    W1 = np.asarray(W1, np.float32)
    shared = {
        "w0x": _wprep(W0[:, :IN]),
        "w0h": _wprep(W0[:, IN:]),
        "w1x": _wprep(W1[:, :H]),
        "w1h": _wprep(W1[:, H:]),
        "b0r": np.asarray(b0, np.float32).reshape(1, G).astype(BFNP),
        "b1r": np.asarray(b1, np.float32).reshape(1, G).astype(BFNP),
    }
    in_maps = []
    for i in range(NCORES):
        xl = x[:, i * BL:(i + 1) * BL, :]                 # [seq, BL, IN]
        a = xl.reshape(nch, T, BL, KC, 128).transpose(0, 4, 3, 1, 2)
        a = np.ascontiguousarray(a).reshape(nch * 128, KC, TB).astype(BFNP)
        xtb = np.zeros(((nch + 1) * 128, KC, TB), BFNP)
        xtb[:nch * 128] = a
        in_maps.append({"xtb": xtb, **shared})
    return in_maps


def assemble_outputs(results, seq=SEQ):
    outputs = np.empty((seq, B, H), np.float32)
    h_n = np.empty((2, B, H), np.float32)
    c_n = np.empty((2, B, H), np.float32)
    for i in range(NCORES):
        sl = slice(i * BL, (i + 1) * BL)
        outputs[:, sl, :] = results[i]["out"].reshape(seq, BL, H)
        fin = results[i]["fin"].reshape(4, 128, KC, BL)
        # fin[j][p, kc, b] -> [b, kc*128+p]
        def unT(j):
            return fin[j].transpose(1, 0, 2).reshape(H, BL).T
        h_n[0, sl] = unT(0)
        h_n[1, sl] = unT(1)
        c_n[0, sl] = unT(2)
        c_n[1, sl] = unT(3)
    return outputs, (h_n, c_n)


_NC_CACHE = None


def kernel(x, W0, b0, W1, b1):
    global _NC_CACHE
    if _NC_CACHE is None:
        _NC_CACHE = build_program()
    in_maps = prep_inputs(x, W0, b0, W1, b1)
    res = run_bass_kernel_spmd(_NC_CACHE, in_maps, list(range(NCORES)))
    return assemble_outputs(res.results)


if __name__ == "__main__":
    rng = np.random.default_rng(0)
    x = rng.standard_normal((SEQ, B, IN), dtype=np.float32)
    s0 = 1.0 / np.sqrt(IN + H)
    s1 = 1.0 / np.sqrt(H + H)
    W0 = rng.uniform(-s0, s0, (G, IN + H)).astype(np.float32)
    b0 = rng.uniform(-s0, s0, G).astype(np.float32)
    W1 = rng.uniform(-s1, s1, (G, H + H)).astype(np.float32)
    b1 = rng.uniform(-s1, s1, G).astype(np.float32)
    out, (h_n, c_n) = kernel(x, W0, b0, W1, b1)
    print("ok", out.shape, h_n.shape, c_n.shape, out[0, 0, :4])


# revision 5
# speedup vs baseline: 2.4017x; 2.4017x over previous
"""Trainium2 Bass kernel: 2-layer LSTM (SEQ=1024, B=64, IN=H=512).

Sharding: data-parallel over batch across 8 NeuronCores (8 rows each);
weights replicated. Per core the kernel runs a chunked pipeline:

  - x-part gate pre-activations Z0x = W0x @ x^T + b0 are computed in bulk
    per chunk of T=16 timesteps (moving operand = x^T chunk, full PE width).
  - the sequential recurrence computes z^T = W_h @ h^T per step with the
    weight tiles stationary and h^T (128x8) moving, so the gate
    pre-activations land transposed ([gate, batch]) in PSUM, which keeps
    all elementwise work on 128 partitions.
  - layer-1's x-part (W1x @ h0) is bulk-computed per chunk as soon as
    layer 0 finishes the chunk; layer 1 runs one chunk behind layer 0 so
    its matmuls fill the PE while layer 0's elementwise chain runs.
  - h1 chunks are transposed back via the PE and DMA'd out contiguously.

All matmul operands are bf16 (fp32 PSUM accumulation); the cell state and
all elementwise math stay fp32.
"""

import sys

if "/opt/trn_rl_repo" not in sys.path:
    sys.path.insert(0, "/opt/trn_rl_repo")

import numpy as np
import ml_dtypes

import concourse.bass as bass
import concourse.bacc as bacc
import concourse.mybir as mybir
from concourse.tile import TileContext
from concourse.masks import make_identity
from concourse.bass import ts
from concourse.bass_utils import run_bass_kernel_spmd

SEQ, B, IN, H = 1024, 64, 512, 512
NCORES = 8
BL = B // NCORES          # batch rows per core
T = 16                    # timesteps per chunk
G = 4 * H                 # stacked gate dim (i,f,o,g)
KC = IN // 128            # contraction chunks of 128
MC = G // 128             # gate-dim chunks of 128
TB = T * BL               # columns per chunk (t-major, batch minor)
F32 = mybir.dt.float32
BF16 = mybir.dt.bfloat16
BFNP = ml_dtypes.bfloat16
AF = mybir.ActivationFunctionType


def build_program(seq=SEQ, reps=1, mc_use=MC):
    nch = seq // T
    nc = bacc.Bacc("TRN2", target_bir_lowering=False, debug=False,
                   num_devices=NCORES)

    xtb_d = nc.dram_tensor("xtb", [(nch + 1) * 128, KC, TB], BF16,
                           kind="ExternalInput")
    w_d = {
        name: nc.dram_tensor(name, [128, KC, G], BF16, kind="ExternalInput")
        for name in ("w0x", "w0h", "w1x", "w1h")
    }
    b0_d = nc.dram_tensor("b0r", [1, G], BF16, kind="ExternalInput")
    b1_d = nc.dram_tensor("b1r", [1, G], BF16, kind="ExternalInput")
    out_d = nc.dram_tensor("out", [seq * BL, H], F32, kind="ExternalOutput")
    fin_d = nc.dram_tensor("fin", [4 * 128, KC, BL], F32, kind="ExternalOutput")

    with TileContext(nc) as tc:
        with (
            tc.tile_pool(name="pers", bufs=1) as pers,
            tc.tile_pool(name="xtp", bufs=2) as xtp,
            tc.tile_pool(name="ewp", bufs=2) as ewp,
            tc.tile_pool(name="zstp", bufs=2, space="PSUM") as zstp,
            tc.tile_pool(name="blkp", bufs=1, space="PSUM") as blkp,
            tc.tile_pool(name="tpp", bufs=1, space="PSUM") as tpp,
            tc.tile_pool(name="outbp", bufs=2) as outbp,
        ):
            # --- persistent SBUF state ---
            w_s = {}
            for name in ("w0x", "w0h", "w1x", "w1h"):
                w = pers.tile([128, KC, G], BF16, name=name + "_s")
                nc.sync.dma_start(out=w, in_=w_d[name].ap())
                w_s[name] = w
            b0r = pers.tile([1, G], BF16)
            nc.sync.dma_start(out=b0r, in_=b0_d.ap())
            b1r = pers.tile([1, G], BF16)
            nc.sync.dma_start(out=b1r, in_=b1_d.ap())
            ones = pers.tile([1, TB], BF16)
            nc.gpsimd.memset(ones, 1.0)
            ident = pers.tile([128, 128], F32)
            make_identity(nc, ident)

            z0x = pers.tile([128, MC, TB], F32)   # layer0 x-part + bias, chunk
            z1x = pers.tile([128, MC, TB], F32)   # layer1 x-part + bias, chunk
            # rolling h^T chunks (bf16, slot 0 = carry from previous chunk)
            h0r = pers.tile([128, KC, (T + 1) * BL], BF16)
            h1r = pers.tile([128, KC, (T + 1) * BL], BF16)
            h1c = pers.tile([128, KC, TB], F32)   # layer1 h chunk (fp32, for out)
            h0f = pers.tile([128, KC, BL], F32)   # layer0 h state (fp32)
            c0 = pers.tile([128, KC, BL], F32)
            c1 = pers.tile([128, KC, BL], F32)

            def load_xt(ci):
                xt = xtp.tile([128, KC, TB], BF16, tag="xt", name="xt")
                nc.sync.dma_start(out=xt, in_=xtb_d.ap()[ts(ci, 128), :, :])
                return xt

            def bulk(zx, w, bias_r, mov):
                # zx[:, m, n] = sum_k w[k, m] * mov[k, n] + bias[m]
                for half in range(2):
                    bp = blkp.tile([128, 8, TB], F32, tag="bp", name="bp")
                    for m8 in range(8):
                        m = half * 8 + m8
                        for k in range(KC):
                            nc.tensor.matmul(
                                bp[:, m8, :],
                                lhsT=w[:, k, m * 128:(m + 1) * 128],
                                rhs=mov[:, k, :],
                                start=(k == 0), stop=False)
                        nc.tensor.matmul(
                            bp[:, m8, :],
                            lhsT=bias_r[0:1, m * 128:(m + 1) * 128],
                            rhs=ones[0:1, :],
                            start=False, stop=True)
                    nc.vector.tensor_copy(zx[:, half * 8:(half + 1) * 8, :], bp)

            def cell(layer, t):
                wh = w_s["w0h"] if layer == 0 else w_s["w1h"]
                hr = h0r if layer == 0 else h1r
                zx = z0x if layer == 0 else z1x
                cst = c0 if layer == 0 else c1
                zp = zstp.tile([128, MC, BL], F32, tag=f"zp{layer}", name="zp")
                for m in range(mc_use):
                    for k in range(KC):
                        nc.tensor.matmul(
                            zp[:, m, :],
                            lhsT=wh[:, k, m * 128:(m + 1) * 128],
                            rhs=hr[:, k, t * BL:(t + 1) * BL],
                            start=(k == 0), stop=(k == KC - 1))
                zf = ewp.tile([128, MC, BL], F32, tag=f"zf{layer}", name="zf")
                nc.vector.tensor_add(zf, zp, zx[:, :, t * BL:(t + 1) * BL])
                sg = ewp.tile([128, 12, BL], F32, tag=f"sg{layer}", name="sg")
                nc.scalar.activation(sg, zf[:, 0:12, :], AF.Sigmoid)
                tg = ewp.tile([128, KC, BL], F32, tag=f"tg{layer}", name="tg")
                nc.scalar.activation(tg, zf[:, 12:16, :], AF.Tanh)
                ig = ewp.tile([128, KC, BL], F32, tag=f"ig{layer}", name="ig")
                nc.vector.tensor_mul(ig, sg[:, 0:4, :], tg)
                fc = ewp.tile([128, KC, BL], F32, tag=f"fc{layer}", name="fc")
                nc.vector.tensor_mul(fc, sg[:, 4:8, :], cst)
                nc.vector.tensor_add(cst, ig, fc)
                th = ewp.tile([128, KC, BL], F32, tag=f"th{layer}", name="th")
                nc.scalar.activation(th, cst, AF.Tanh)
                if layer == 0:
                    nc.vector.tensor_mul(h0f, sg[:, 8:12, :], th)
                    nc.vector.tensor_copy(hr[:, :, (t + 1) * BL:(t + 2) * BL], h0f)
                else:
                    hslice = h1c[:, :, t * BL:(t + 1) * BL]
                    nc.vector.tensor_mul(hslice, sg[:, 8:12, :], th)
                    nc.vector.tensor_copy(hr[:, :, (t + 1) * BL:(t + 2) * BL],
                                          hslice)

            def roll(hr):
                nc.vector.tensor_copy(hr[:, :, 0:BL],
                                      hr[:, :, T * BL:(T + 1) * BL])

            def flush_out(ci):
                tp = tpp.tile([128, H], F32, tag="tp", name="tp")
                for k in range(KC):
                    nc.tensor.transpose(tp[:, k * 128:(k + 1) * 128],
                                        h1c[:, k, :], ident)
                ob = outbp.tile([128, H], F32, tag="ob", name="ob")
                nc.vector.tensor_copy(ob, tp)
                nc.sync.dma_start(out=out_d.ap()[ts(ci, 128), :], in_=ob)

            import contextlib
            rep_ctx = tc.For_i(0, reps) if reps > 1 else contextlib.nullcontext()
            with rep_ctx:
                nc.gpsimd.memset(h0r, 0.0)
                nc.gpsimd.memset(h1r, 0.0)
                nc.gpsimd.memset(h0f, 0.0)
                nc.gpsimd.memset(c0, 0.0)
                nc.gpsimd.memset(c1, 0.0)

                # --- prologue: chunk 0 of layer 0 ---
                xt = load_xt(0)
                bulk(z0x, w_s["w0x"], b0r, xt)
                for t in range(T):
                    cell(0, t)
                bulk(z1x, w_s["w1x"], b1r, h0r[:, :, BL:])
                roll(h0r)
                xt = load_xt(1)
                bulk(z0x, w_s["w0x"], b0r, xt)

                # --- main loop: layer0 chunk c + layer1 chunk c-1 ---
                with tc.For_i(1, nch) as c:
                    xt = load_xt(c + 1)
                    for t in range(T):
                        cell(0, t)
                        cell(1, t)
                    flush_out(c - 1)
                    bulk(z1x, w_s["w1x"], b1r, h0r[:, :, BL:])
                    roll(h0r)
                    roll(h1r)
                    bulk(z0x, w_s["w0x"], b0r, xt)

                # --- epilogue: layer 1, last chunk ---
                for t in range(T):
                    cell(1, t)
                flush_out(nch - 1)
                nc.sync.dma_start(out=fin_d.ap()[0:128, :, :], in_=h0f)
                nc.sync.dma_start(out=fin_d.ap()[128:256, :, :],
                                  in_=h1c[:, :, (T - 1) * BL:T * BL])
                nc.sync.dma_start(out=fin_d.ap()[256:384, :, :], in_=c0)
                nc.sync.dma_start(out=fin_d.ap()[384:512, :, :], in_=c1)

    nc.compile()
    return nc


def _wprep(wpart):
    # [G, 512] -> [p, kc, m] with w[p, kc, m] = wpart[m, kc*128+p]
    g = wpart.shape[0]
    return np.ascontiguousarray(
        wpart.T.reshape(KC, 128, g).transpose(1, 0, 2)).astype(BFNP)


def prep_inputs(x, W0, b0, W1, b1, seq=SEQ):
    nch = seq // T
    x = np.asarray(x, np.float32)
    W0 = np.asarray(W0, np.float32)
    W1 = np.asarray(W1, np.float32)
    shared = {
        "w0x": _wprep(W0[:, :IN]),
        "w0h": _wprep(W0[:, IN:]),
        "w1x": _wprep(W1[:, :H]),
        "w1h": _wprep(W1[:, H:]),
        "b0r": np.asarray(b0, np.float32).reshape(1, G).astype(BFNP),
        "b1r": np.asarray(b1, np.float32).reshape(1, G).astype(BFNP),
    }
    in_maps = []
    for i in range(NCORES):
        xl = x[:, i * BL:(i + 1) * BL, :]                 # [seq, BL, IN]
        a = xl.reshape(nch, T, BL, KC, 128).transpose(0, 4, 3, 1, 2)
        a = np.ascontiguousarray(a).reshape(nch * 128, KC, TB).astype(BFNP)
        xtb = np.zeros(((nch + 1) * 128, KC, TB), BFNP)
        xtb[:nch * 128] = a
        in_maps.append({"xtb": xtb, **shared})
    return in_maps


def assemble_outputs(results, seq=SEQ):
    outputs = np.empty((seq, B, H), np.float32)
    h_n = np.empty((2, B, H), np.float32)
    c_n = np.empty((2, B, H), np.float32)
    for i in range(NCORES):
        sl = slice(i * BL, (i + 1) * BL)
        outputs[:, sl, :] = results[i]["out"].reshape(seq, BL, H)
        fin = results[i]["fin"].reshape(4, 128, KC, BL)
        # fin[j][p, kc, b] -> [b, kc*128+p]
        def unT(j):
            return fin[j].transpose(1, 0, 2).reshape(H, BL).T
        h_n[0, sl] = unT(0)
        h_n[1, sl] = unT(1)
        c_n[0, sl] = unT(2)
        c_n[1, sl] = unT(3)
    return outputs, (h_n, c_n)


_NC_CACHE = None


def kernel(x, W0, b0, W1, b1):
    global _NC_CACHE
    if _NC_CACHE is None:
        _NC_CACHE = build_program()
    in_maps = prep_inputs(x, W0, b0, W1, b1)
    res = run_bass_kernel_spmd(_NC_CACHE, in_maps, list(range(NCORES)))
    return assemble_outputs(res.results)


if __name__ == "__main__":
    rng = np.random.default_rng(0)
    x = rng.standard_normal((SEQ, B, IN), dtype=np.float32)
    s0 = 1.0 / np.sqrt(IN + H)
    s1 = 1.0 / np.sqrt(H + H)
    W0 = rng.uniform(-s0, s0, (G, IN + H)).astype(np.float32)
    b0 = rng.uniform(-s0, s0, G).astype(np.float32)
    W1 = rng.uniform(-s1, s1, (G, H + H)).astype(np.float32)
    b1 = rng.uniform(-s1, s1, G).astype(np.float32)
    out, (h_n, c_n) = kernel(x, W0, b0, W1, b1)
    print("ok", out.shape, h_n.shape, c_n.shape, out[0, 0, :4])
